# revision 5
# baseline (speedup 1.0000x reference)
"""AP-loss (average-precision ranking loss) on 8 Trainium2 NeuronCores.

Math
----
The reference scans the 256 sorted foreground logits f_i and, per step,
computes
    a_i = sum_fg clip((f_j - f_i)/2 + 1/2, 0, 1) + 1/2
    b_i = sum_bg clip((x  - f_i)/2 + 1/2, 0, 1)
    cur_i = a_i / (a_i + b_i);  loss = 1 - mean(runningmax(cur)).
Since clip((x-f)/2+1/2, 0, 1) = [relu(x - (f-1)) - relu(x - (f+1))] / 2,
every b_i is a difference of the single convex function
    g(t) = sum_bg relu(x - t)
evaluated at the two points f_i -+ 1.  g has curvature = local data density,
so it is extremely smooth at scale (range/K): we sample g on a K-point
uniform grid covering [min f - 1, max f + 1] (exact per-element relu sums on
device) and evaluate g(f_i -+ 1) by cubic Hermite interpolation.  The
interpolation is a fixed linear map of the K samples, so the host (which
knows the thresholds — the replicated "small fg subset" of the sharding
hint) bakes it into a [K, 256] matrix M with b = g @ M.  Measured accuracy
of this scheme (K=32) vs the exact scan: max relative error on b ~1e-3,
relative error on the loss ~4e-9.

Distribution (data-parallel, per sharding hint)
-----------------------------------------------
The flat 2M logits/targets axis is sharded 8 ways; each core computes
partial g samples over its shard (the per-step clip+partial-sum of the
hint, batched over all steps); one AllReduce of the K-vector replaces the
per-step psums; the small fg-derived tensors (grid, M, fg values) are
replicated.  Every core then finishes the tiny 256-step tail redundantly
and writes the same scalar loss.

Per-core pipeline: DMA shard -> mask bg (x - 1e4*t, bf16) ->
K relu+sum passes split between ScalarE (fused accum_out) and
VectorE+TensorE (tensor_scalar relu tile, ones-matmul column sums) ->
PE partition reduce -> AllReduce[K] -> b = g@M (PE) -> a (exact, PE) ->
cur -> running max (tensor_tensor_scan) -> loss.
"""

import numpy as np
import ml_dtypes

import concourse.bass as bass
import concourse.bacc as bacc
import concourse.mybir as mybir
import concourse.tile as tile
from concourse.bass_utils import run_bass_kernel_spmd

F32 = mybir.dt.float32
BF16 = mybir.dt.bfloat16
ALU = mybir.AluOpType
AXL = mybir.AxisListType
ACT_FN = mybir.ActivationFunctionType

N_CORES = 8
P = 128           # SBUF partitions
W = 1956          # free-dim elements per partition (8*128*1956 >= 2e6), mult of 4
WCHUNK = 489      # PE moving chunk (4 chunks of 489 = 1956, <= 512)
FGPAD = 256       # padded fg count
K = 32            # g-sample grid points
D_DVE = 18        # grid points computed on VectorE+TensorE; rest on ScalarE
NEG = -1e4        # bg-mask shift
DELTA = 1.0


def _build_nc():
    nc = bacc.Bacc(trn_type=None, target_bir_lowering=False)

    xb = nc.declare_dram_parameter("xb", [P, W], BF16, isOutput=False)
    tb = nc.declare_dram_parameter("tb", [P, W], BF16, isOutput=False)
    grid = nc.declare_dram_parameter("grid", [P, K], F32, isOutput=False)
    gridneg = nc.declare_dram_parameter("gridneg", [P, K], F32, isOutput=False)
    fgrow = nc.declare_dram_parameter("fgrow", [P, FGPAD], F32, isOutput=False)
    fgcol = nc.declare_dram_parameter("fgcol", [P, 2], F32, isOutput=False)
    mmov = nc.declare_dram_parameter("mmov", [K, FGPAD], F32, isOutput=False)
    valid = nc.declare_dram_parameter("valid", [1, FGPAD], F32, isOutput=False)
    invden = nc.declare_dram_parameter("invden", [1, 1], F32, isOutput=False)
    out = nc.declare_dram_parameter("out", [1, 1], F32, isOutput=True)

    with tile.TileContext(nc) as tc:
        with (
            tc.tile_pool(name="big", bufs=1) as big,
            tc.tile_pool(name="rbuf", bufs=2) as rbuf,
            tc.tile_pool(name="small", bufs=1) as small,
            tc.tile_pool(name="psum", bufs=1, space="PSUM") as psum,
            tc.tile_pool(name="dram", bufs=1, space="DRAM") as dram,
        ):
            # ---- DMA inputs ----
            xb_s = big.tile([P, W], BF16, tag="xb_s")
            tb_s = big.tile([P, W], BF16, tag="tb_s")
            nc.sync.dma_start(xb_s[:], xb[:])
            nc.sync.dma_start(tb_s[:], tb[:])

            grid_s = small.tile([P, K], F32, tag="grid_s")
            gridneg_s = small.tile([P, K], F32, tag="gridneg_s")
            fgrow_s = small.tile([P, FGPAD], F32, tag="fgrow_s")
            fgcol_s = small.tile([P, 2], F32, tag="fgcol_s")
            mmov_s = small.tile([K, FGPAD], F32, tag="mmov_s")
            valid_s = small.tile([1, FGPAD], F32, tag="valid_s")
            invden_s = small.tile([1, 1], F32, tag="invden_s")
            nc.sync.dma_start(grid_s[:], grid[:])
            nc.sync.dma_start(gridneg_s[:], gridneg[:])
            nc.sync.dma_start(fgrow_s[:], fgrow[:])
            nc.sync.dma_start(fgcol_s[:], fgcol[:])
            nc.sync.dma_start(mmov_s[:], mmov[:])
            nc.sync.dma_start(valid_s[:], valid[:])
            nc.sync.dma_start(invden_s[:], invden[:])

            ones_b = small.tile([P, 1], BF16, tag="ones_b")
            ones_f = small.tile([P, 1], F32, tag="ones_f")
            nc.vector.memset(ones_b[:], 1.0)
            nc.vector.memset(ones_f[:], 1.0)

            # ---- mask background: xm = x + NEG*t (bf16) ----
            tneg = big.tile([P, W], BF16, tag="tneg")
            xm = big.tile([P, W], BF16, tag="xm")
            nc.vector.tensor_scalar(tneg[:], tb_s[:], float(NEG), None, ALU.mult)
            nc.vector.tensor_tensor(xm[:], xb_s[:], tneg[:], ALU.add)

            # ---- K relu+sum passes ----
            # ScalarE partials land in gacc columns; VectorE partials go
            # through TensorE ones-matmuls into psum_g rows.
            gacc = small.tile([P, K], F32, tag="gacc")
            nc.vector.memset(gacc[:], 0.0)
            psum_g = psum.tile([D_DVE, 512], F32, tag="psum_g")

            # eye_blk[:, m*D : (m+1)*D] is the all-partition indicator of
            # column m — a stationary that routes point m's column sums to
            # PSUM row m (PE requires out base partition in {0, 32, 64}).
            eye_blk = small.tile([P, D_DVE * D_DVE], BF16, tag="eye_blk")
            nc.vector.memset(eye_blk[:], 0.0)
            for m in range(D_DVE):
                nc.vector.memset(
                    eye_blk[:, m * D_DVE + m : m * D_DVE + m + 1], 1.0
                )

            for m in range(D_DVE):
                r = rbuf.tile([P, W], BF16, tag="r")
                nc.vector.tensor_scalar(
                    r[:], xm[:], grid_s[:, m : m + 1], 0.0, ALU.subtract, ALU.max
                )
                for c in range(4):
                    nc.tensor.matmul(
                        psum_g[:, 0:WCHUNK],
                        eye_blk[:, m * D_DVE : (m + 1) * D_DVE],
                        r[:, c * WCHUNK : (c + 1) * WCHUNK],
                        start=(m == 0 and c == 0),
                        stop=(m == D_DVE - 1 and c == 3),
                        skip_group_check=True,
                    )
            act_scratch = big.tile([P, W], BF16, tag="act_scratch")
            for m in range(D_DVE, K):
                nc.scalar.activation(
                    act_scratch[:],
                    xm[:],
                    ACT_FN.Relu,
                    bias=gridneg_s[:, m : m + 1],
                    scale=1.0,
                    accum_out=gacc[:, m : m + 1],
                )

            # ---- combine partials into g_col [K, 1] ----
            gcol = small.tile([K, 1], F32, tag="gcol")
            nc.vector.memset(gcol[:], 0.0)
            nc.vector.tensor_reduce(
                gcol[0:D_DVE, 0:1], psum_g[0:D_DVE, 0:WCHUNK], AXL.X, ALU.add
            )
            psum_ga = psum.tile([K, 1], F32, tag="psum_ga")
            nc.tensor.matmul(psum_ga[:], gacc[:], ones_f[:], start=True, stop=True)
            gsum = small.tile([K, 1], F32, tag="gsum")
            nc.vector.tensor_tensor(gsum[:], gcol[:], psum_ga[:], ALU.add)

            # ---- AllReduce g across the 8 shards ----
            gin_d = dram.tile([K, 1], F32, tag="gin_d")
            gout_d = dram.tile([K, 1], F32, tag="gout_d")
            nc.sync.dma_start(gin_d[:], gsum[:])
            nc.gpsimd.collective_compute(
                "AllReduce",
                ALU.add,
                replica_groups=[list(range(N_CORES))],
                ins=[gin_d.opt()],
                outs=[gout_d.opt()],
            )
            gfull = small.tile([K, 1], F32, tag="gfull")
            nc.sync.dma_start(gfull[:], gout_d[:])

            # ---- b row: b[1, 256] = gfull^T @ M ----
            psum_b = psum.tile([1, FGPAD], F32, tag="psum_b")
            nc.tensor.matmul(psum_b[:], gfull[:], mmov_s[:], start=True, stop=True)

            # ---- a row: a = (256 - sum_j clip((f_i-f_j)/2+1/2)) + 1/2 ----
            psum_a = psum.tile([1, FGPAD], F32, tag="psum_a")
            for c in range(2):
                u1 = small.tile([P, FGPAD], F32, tag="u1")
                nc.vector.tensor_scalar(
                    u1[:], fgrow_s[:], fgcol_s[:, c : c + 1], 0.5,
                    ALU.subtract, ALU.mult,
                )
                nc.vector.tensor_scalar(u1[:], u1[:], 0.5, 0.0, ALU.add, ALU.max)
                nc.vector.tensor_scalar(u1[:], u1[:], 1.0, None, ALU.min)
                nc.tensor.matmul(
                    psum_a[:], ones_f[:], u1[:], start=(c == 0), stop=(c == 1)
                )
            a_row = small.tile([1, FGPAD], F32, tag="a_row")
            nc.vector.tensor_scalar(
                a_row[:], psum_a[:], float(FGPAD) + 0.5, -1.0, ALU.subtract, ALU.mult
            )

            # ---- cur = a/(a+b), running max, sum, loss ----
            den = small.tile([1, FGPAD], F32, tag="den")
            nc.vector.tensor_tensor(den[:], a_row[:], psum_b[:], ALU.add)
            rec = small.tile([1, FGPAD], F32, tag="rec")
            nc.vector.reciprocal(rec[:], den[:])
            cur = small.tile([1, FGPAD], F32, tag="cur")
            nc.vector.tensor_tensor(cur[:], a_row[:], rec[:], ALU.mult)
            nc.vector.tensor_tensor(cur[:], cur[:], valid_s[:], ALU.mult)
            prec = small.tile([1, FGPAD], F32, tag="prec")
            nc.vector.tensor_tensor_scan(
                prec[:], cur[:], cur[:], 0.0, ALU.max, ALU.max
            )
            psum_p = small.tile([1, 1], F32, tag="psum_p")
            nc.vector.tensor_reduce(psum_p[:], prec[:], AXL.X, ALU.add)
            loss_t = small.tile([1, 1], F32, tag="loss_t")
            nc.vector.tensor_scalar(
                loss_t[:], psum_p[:], invden_s[0:1, 0:1], None, ALU.mult
            )
            nc.vector.tensor_scalar(
                loss_t[:], loss_t[:], -1.0, 1.0, ALU.mult, ALU.add
            )
            nc.sync.dma_start(out[:], loss_t[:])

    nc.compile()
    return nc


def _hermite_weight_rows(taus, lo, h, K):
    """Cardinal cubic-Hermite weights: row r of the result W satisfies
    p(taus[r]) = W[r] @ g for g sampled on the uniform grid lo + h*[0..K)."""
    W = np.zeros((len(taus), K), dtype=np.float64)
    t = (np.asarray(taus, dtype=np.float64) - lo) / h
    c = np.clip(np.floor(t).astype(np.int64), 0, K - 2)
    u = t - c
    h00 = 2 * u**3 - 3 * u**2 + 1
    h10 = u**3 - 2 * u**2 + u
    h01 = -2 * u**3 + 3 * u**2
    h11 = u**3 - u**2
    rows = np.arange(len(taus))
    np.add.at(W, (rows, c), h00)
    np.add.at(W, (rows, c + 1), h01)
    # derivative weights: central differences, one-sided at the ends
    for coeff, idx in ((h10, c), (h11, c + 1)):
        left = np.where(idx == 0, 0, idx - 1)
        right = np.where(idx == K - 1, K - 1, idx + 1)
        scale = np.where((idx == 0) | (idx == K - 1), 1.0, 0.5)
        np.add.at(W, (rows, right), coeff * scale)
        np.add.at(W, (rows, left), -coeff * scale)
    return W


def kernel(logits, targets, fg_num):
    logits = np.asarray(logits, dtype=np.float32).reshape(-1)
    targets = np.asarray(targets, dtype=np.int32).reshape(-1)
    fgn = int(np.asarray(fg_num))
    n = logits.shape[0]
    assert n == 2_000_000, f"kernel hardcoded for N=2e6, got {n}"

    if fgn <= 0:
        return np.array([1.0], dtype=np.float32)

    # foreground subset (replicated to all shards, per the sharding hint);
    # mirrors jnp.nonzero(targets == 1, size=fg_num, fill_value=0)
    idx = np.flatnonzero(targets == 1)[:fgn]
    if idx.size < fgn:
        idx = np.concatenate([idx, np.zeros(fgn - idx.size, dtype=idx.dtype)])
    f_sorted = np.sort(logits[idx].astype(np.float64))

    lo = f_sorted[0] - DELTA
    hi = f_sorted[-1] + DELTA
    h = max((hi - lo) / (K - 1), 1e-6)
    gridv = (lo + h * np.arange(K)).astype(np.float64)

    wm = _hermite_weight_rows(f_sorted - DELTA, lo, h, K) - _hermite_weight_rows(
        f_sorted + DELTA, lo, h, K
    )
    M = np.zeros((K, FGPAD), dtype=np.float32)
    M[:, :fgn] = 0.5 * wm.T

    fg_pad = np.full(FGPAD, NEG, dtype=np.float32)
    fg_pad[:fgn] = f_sorted.astype(np.float32)
    validv = np.zeros((1, FGPAD), dtype=np.float32)
    validv[0, :fgn] = 1.0

    # shard the flat axis 8 ways, pad tail with masked-out elements
    total = N_CORES * P * W
    xpad = np.zeros(total, dtype=np.float32)
    xpad[:n] = logits
    tpad = np.ones(total, dtype=np.float32)
    tpad[:n] = (targets != 0).astype(np.float32)
    xsh = xpad.reshape(N_CORES, P, W).astype(ml_dtypes.bfloat16)
    tsh = tpad.reshape(N_CORES, P, W).astype(ml_dtypes.bfloat16)

    grid_t = np.broadcast_to(
        gridv.astype(np.float32), (P, K)
    ).copy()
    gridneg_t = (-grid_t).copy()
    fgrow_t = np.broadcast_to(fg_pad, (P, FGPAD)).copy()
    fgcol_t = fg_pad.reshape(2, P).T.copy()
    invden_t = np.array([[1.0 / max(fgn, 1)]], dtype=np.float32)

    in_maps = []
    for c in range(N_CORES):
        in_maps.append(
            {
                "xb": xsh[c],
                "tb": tsh[c],
                "grid": grid_t,
                "gridneg": gridneg_t,
                "fgrow": fgrow_t,
                "fgcol": fgcol_t,
                "mmov": M,
                "valid": validv,
                "invden": invden_t,
            }
        )

    nc = _build_nc()
    import os

    trace = bool(int(os.environ.get("APLOSS_TRACE", "0")))
    res = run_bass_kernel_spmd(
        nc, in_maps, core_ids=list(range(N_CORES)), trace=trace
    )
    global _last_results
    _last_results = res
    loss = np.asarray(res.results[0]["out"]).reshape(1).astype(np.float32)
    return loss


_last_results = None


if __name__ == "__main__":
    rng = np.random.default_rng(0)
    x = rng.standard_normal(2_000_000).astype(np.float32)
    t = np.zeros(2_000_000, dtype=np.int32)
    t[rng.choice(2_000_000, 256, replace=False)] = 1
    print(kernel(logits=x, targets=t, fg_num=256))
